# revision 32
# baseline (speedup 1.0000x reference)
"""Discriminator-loss kernel for Trainium2, SPMD across 8 NeuronCores.

Computes mean(where(s == other_s, 1, -1) * x) for N = 2^25 elements.

Strategy (data-parallel, per the sharding hint): each core streams its
1/8 shard of (s, other_s, x) from HBM and reduces it with two DVE ops
per compute sub-tile:
    eq   = is_equal(s, other_s)            # int32 -> f32 {0.0, 1.0}
    prod = (eq - 0.5) * x                  # = +-x/2, exact in f32
    acc[:, k] = sum_freeaxis(prod)         # fused accum of the same op
Middle tiles use 2 MiB DMAs (HBM efficiency); the first and last tile
are tapered into 512 KiB sub-DMAs so the pipeline fills fast at the
start and the final compute quantum gates on a small DMA at the end.
Per-core output is the [128, n_cols] grid of partial sums of (+-x/2);
the host sums the partials in float64 and multiplies by 2/N.
"""

import contextlib
import ctypes
import os
import sys
import types

import numpy as np


def _install_ntff_hook_shim():
    """Register the axon NTFF-profile hook if the image's ``antenv`` lacks
    ``axon_hooks`` (boot degrades silently in that case, which breaks
    ``run_bass_kernel_spmd(trace=True)``). Same ctypes recipe as
    ``trn_agent_boot.trn_boot._ntff_profile_via_ctypes``. No-op when the
    module already exists or the .so is absent."""
    try:
        import antenv.axon_hooks  # noqa: F401

        return
    except ImportError:
        pass
    try:
        mod = types.ModuleType("antenv.axon_hooks")
        holder = {"hook": None}
        mod.set_axon_ntff_profile_hook = lambda h: holder.__setitem__("hook", h)
        mod.get_axon_ntff_profile_hook = lambda: holder["hook"]
        sys.modules["antenv.axon_hooks"] = mod
        try:
            import antenv

            antenv.axon_hooks = mod
        except ImportError:
            pass

        so_path = "/opt/axon/libaxon_pjrt.so"
        if not os.path.exists(so_path):
            return
        lib = ctypes.CDLL(so_path)
        if not hasattr(lib, "axon_start_nrt_profile"):
            return
        lib.axon_start_nrt_profile.argtypes = [
            ctypes.POINTER(ctypes.c_int64),
            ctypes.c_size_t,
        ]
        lib.axon_start_nrt_profile.restype = ctypes.c_int64
        lib.axon_stop_nrt_profile.argtypes = [ctypes.c_char_p]
        lib.axon_stop_nrt_profile.restype = ctypes.c_int64

        @contextlib.contextmanager
        def _hook(output_dir, device_ids):
            import jax

            jax.devices()
            if device_ids:
                ids = (ctypes.c_int64 * len(device_ids))(*device_ids)
                rc = lib.axon_start_nrt_profile(ids, len(device_ids))
            else:
                rc = lib.axon_start_nrt_profile(None, 0)
            if rc != 0:
                raise RuntimeError(f"axon_start_nrt_profile rc={rc}")
            try:
                yield
            finally:
                n = lib.axon_stop_nrt_profile(str(output_dir).encode())
                print(f"ntff profile: {n} file(s) -> {output_dir}", file=sys.stderr)

        holder["hook"] = _hook
    except Exception:
        pass


_install_ntff_hook_shim()

from concourse import bacc, mybir, tile
from concourse.bass_utils import run_bass_kernel_spmd

N = 33554432
NCORES = 8
PER = N // NCORES  # 4194304 elements per core
P = 128            # SBUF partitions
F = 4096           # free elements per DMA tile (2 MiB f32 tiles)
T = PER // (P * F)  # 8 tiles per tensor per core
FC = 1024          # edge-tile DMA/compute quantum (short pipeline head/tail)
FC_MID = 2048      # compute sub-tile for middle tiles
NSUB = F // FC

_cache = {}


def _build():
    if "nc" in _cache:
        return _cache["nc"]

    nc = bacc.Bacc(
        "TRN2", target_bir_lowering=False, debug=False, num_devices=NCORES
    )

    # One interleaved flat parameter per core: per DMA tile t the host packs
    # [s_t | o_t | x_t] (x bit-punned to int32) at consecutive addresses, so
    # the core's DMA sequence walks a single sequential HBM address range
    # (fewer simultaneously-open banks -> less conflict surface with the
    # HBM-stack pair partner). Order is irrelevant for a global sum. Each
    # tile is a contiguous block viewed as [128, f]
    # (partition p <-> flat [p*f, (p+1)*f)).
    sox = nc.dram_tensor("sox", [3 * PER], mybir.dt.int32, kind="ExternalInput")
    out_cols = 2 * NSUB + (T - 2) * (F // FC_MID)
    out = nc.dram_tensor(
        "out", [P, out_cols], mybir.dt.float32, kind="ExternalOutput"
    )

    def view(lo, f):
        return sox.ap()[lo : lo + P * f].rearrange("(p f) -> p f", p=P)

    with tile.TileContext(nc) as tc:
        with (
            tc.tile_pool(name="io", bufs=2) as io_pool,
            tc.tile_pool(name="edge", bufs=6) as edge_pool,
            tc.tile_pool(name="work", bufs=2) as work_pool,
            tc.tile_pool(name="stat", bufs=1) as stat_pool,
        ):
            acc = stat_pool.tile([P, out_cols], mybir.dt.float32)
            col_counter = [0]

            def compute(s_ap, o_ap, x_ap, fc):
                col = col_counter[0]
                col_counter[0] += 1
                eq = work_pool.tile([P, fc], mybir.dt.float32, tag="eq")
                nc.vector.tensor_tensor(
                    out=eq[:], in0=s_ap, in1=o_ap, op=mybir.AluOpType.is_equal
                )
                nc.vector.scalar_tensor_tensor(
                    out=eq[:],
                    in0=eq[:],
                    scalar=-0.5,
                    in1=x_ap,
                    op0=mybir.AluOpType.add,
                    op1=mybir.AluOpType.mult,
                    accum_out=acc[:, col : col + 1],
                )

            for t in range(T):
                base = 3 * t * P * F
                if t == 0 or t == T - 1:
                    # Tapered edge tiles: 512 KiB sub-DMAs so the pipeline
                    # fills fast at the start and the last compute quantum
                    # gates on a small DMA at the end.
                    for j in range(NSUB):
                        lo = base + j * P * FC
                        s_t = edge_pool.tile([P, FC], mybir.dt.int32, tag="se")
                        o_t = edge_pool.tile([P, FC], mybir.dt.int32, tag="oe")
                        x_t = edge_pool.tile([P, FC], mybir.dt.float32, tag="xe")
                        nc.sync.dma_start(out=s_t[:], in_=view(lo, FC))
                        nc.sync.dma_start(out=o_t[:], in_=view(lo + P * F, FC))
                        nc.sync.dma_start(
                            out=x_t[:].bitcast(mybir.dt.int32),
                            in_=view(lo + 2 * P * F, FC),
                        )
                        compute(s_t[:], o_t[:], x_t[:], FC)
                else:
                    s_t = io_pool.tile([P, F], mybir.dt.int32, tag="s")
                    o_t = io_pool.tile([P, F], mybir.dt.int32, tag="o")
                    x_t = io_pool.tile([P, F], mybir.dt.float32, tag="x")
                    nc.sync.dma_start(out=s_t[:], in_=view(base, F))
                    nc.sync.dma_start(out=o_t[:], in_=view(base + P * F, F))
                    nc.sync.dma_start(
                        out=x_t[:].bitcast(mybir.dt.int32),
                        in_=view(base + 2 * P * F, F),
                    )
                    for j in range(F // FC_MID):
                        sl = slice(j * FC_MID, (j + 1) * FC_MID)
                        compute(s_t[:, sl], o_t[:, sl], x_t[:, sl], FC_MID)

            nc.sync.dma_start(out=out[:], in_=acc[:])

    nc.compile()
    _cache["nc"] = nc
    return nc


def _shard_interleaved(s, other_s, x, c):
    """Per-core buffer: for each DMA tile t, consecutive blocks
    [s_t | o_t | x_t] (each P*F int32 words; x bit-punned)."""
    sl = slice(c * PER, (c + 1) * PER)
    return np.ascontiguousarray(
        np.stack(
            [
                s[sl].reshape(T, P * F),
                other_s[sl].reshape(T, P * F),
                x[sl].view(np.int32).reshape(T, P * F),
            ],
            axis=1,
        ).reshape(3 * PER)
    )


def run(s, other_s, x, **spmd_kwargs):
    """Run on HW; returns (full_output, BassKernelResults)."""
    s = np.ascontiguousarray(np.asarray(s, dtype=np.int32).reshape(N))
    other_s = np.ascontiguousarray(np.asarray(other_s, dtype=np.int32).reshape(N))
    x = np.ascontiguousarray(np.asarray(x, dtype=np.float32).reshape(N))

    nc = _build()
    in_maps = [
        {"sox": _shard_interleaved(s, other_s, x, c)} for c in range(NCORES)
    ]
    res = run_bass_kernel_spmd(nc, in_maps, core_ids=list(range(NCORES)), **spmd_kwargs)

    total = 0.0
    for r in res.results:
        total += float(np.sum(r["out"].astype(np.float64)))
    full = np.array(2.0 * total / N, dtype=np.float32)
    return full, res


def kernel(s, other_s, x):
    out, _ = run(s, other_s, x)
    return out


# revision 34
# speedup vs baseline: 1.0439x; 1.0439x over previous
"""Discriminator-loss kernel for Trainium2, SPMD across 8 NeuronCores.

Computes mean(where(s == other_s, 1, -1) * x) for N = 2^25 elements.

Strategy (data-parallel, per the sharding hint): each core streams its
1/8 shard of (s, other_s, x) from HBM and reduces it with two DVE ops
per compute sub-tile:
    eq   = is_equal(s, other_s)            # int32 -> f32 {0.0, 1.0}
    prod = (eq - 0.5) * x                  # = +-x/2, exact in f32
    acc[:, k] = sum_freeaxis(prod)         # fused accum of the same op
Middle tiles use 2 MiB DMAs (HBM efficiency); the first and last tile
are tapered into 512 KiB sub-DMAs so the pipeline fills fast at the
start and the final compute quantum gates on a small DMA at the end.
Per-core output is the [128, n_cols] grid of partial sums of (+-x/2);
the host sums the partials in float64 and multiplies by 2/N.
"""

import contextlib
import ctypes
import os
import sys
import types

import numpy as np


def _install_ntff_hook_shim():
    """Register the axon NTFF-profile hook if the image's ``antenv`` lacks
    ``axon_hooks`` (boot degrades silently in that case, which breaks
    ``run_bass_kernel_spmd(trace=True)``). Same ctypes recipe as
    ``trn_agent_boot.trn_boot._ntff_profile_via_ctypes``. No-op when the
    module already exists or the .so is absent."""
    try:
        import antenv.axon_hooks  # noqa: F401

        return
    except ImportError:
        pass
    try:
        mod = types.ModuleType("antenv.axon_hooks")
        holder = {"hook": None}
        mod.set_axon_ntff_profile_hook = lambda h: holder.__setitem__("hook", h)
        mod.get_axon_ntff_profile_hook = lambda: holder["hook"]
        sys.modules["antenv.axon_hooks"] = mod
        try:
            import antenv

            antenv.axon_hooks = mod
        except ImportError:
            pass

        so_path = "/opt/axon/libaxon_pjrt.so"
        if not os.path.exists(so_path):
            return
        lib = ctypes.CDLL(so_path)
        if not hasattr(lib, "axon_start_nrt_profile"):
            return
        lib.axon_start_nrt_profile.argtypes = [
            ctypes.POINTER(ctypes.c_int64),
            ctypes.c_size_t,
        ]
        lib.axon_start_nrt_profile.restype = ctypes.c_int64
        lib.axon_stop_nrt_profile.argtypes = [ctypes.c_char_p]
        lib.axon_stop_nrt_profile.restype = ctypes.c_int64

        @contextlib.contextmanager
        def _hook(output_dir, device_ids):
            import jax

            jax.devices()
            if device_ids:
                ids = (ctypes.c_int64 * len(device_ids))(*device_ids)
                rc = lib.axon_start_nrt_profile(ids, len(device_ids))
            else:
                rc = lib.axon_start_nrt_profile(None, 0)
            if rc != 0:
                raise RuntimeError(f"axon_start_nrt_profile rc={rc}")
            try:
                yield
            finally:
                n = lib.axon_stop_nrt_profile(str(output_dir).encode())
                print(f"ntff profile: {n} file(s) -> {output_dir}", file=sys.stderr)

        holder["hook"] = _hook
    except Exception:
        pass


_install_ntff_hook_shim()

from concourse import bacc, mybir, tile
from concourse.bass_utils import run_bass_kernel_spmd

N = 33554432
NCORES = 8
PER = N // NCORES  # 4194304 elements per core
P = 128            # SBUF partitions
F = 4096           # free elements per DMA tile (2 MiB f32 tiles)
T = PER // (P * F)  # 8 tiles per tensor per core
FC = 1024          # edge-tile DMA/compute quantum (short pipeline head/tail)
FC_MID = 2048      # compute sub-tile for middle tiles
NSUB = F // FC

_cache = {}


def _build():
    if "nc" in _cache:
        return _cache["nc"]

    nc = bacc.Bacc(
        "TRN2", target_bir_lowering=False, debug=False, num_devices=NCORES
    )

    # One interleaved flat parameter per core: per DMA tile t the host packs
    # [s_t | o_t | x_t] (x bit-punned to int32) at consecutive addresses, so
    # the core's DMA sequence walks a single sequential HBM address range
    # (fewer simultaneously-open banks -> less conflict surface with the
    # HBM-stack pair partner). Order is irrelevant for a global sum. Each
    # tile is a contiguous block viewed as [128, f]
    # (partition p <-> flat [p*f, (p+1)*f)).
    sox = nc.dram_tensor("sox", [3 * PER], mybir.dt.int32, kind="ExternalInput")
    out_cols = 2 * (NSUB + 1) + (T - 2) * (F // FC_MID)
    out = nc.dram_tensor(
        "out", [P, out_cols], mybir.dt.float32, kind="ExternalOutput"
    )

    def view(lo, f):
        return sox.ap()[lo : lo + P * f].rearrange("(p f) -> p f", p=P)

    with tile.TileContext(nc) as tc:
        with (
            tc.tile_pool(name="io", bufs=2) as io_pool,
            tc.tile_pool(name="edge", bufs=6) as edge_pool,
            tc.tile_pool(name="work", bufs=2) as work_pool,
            tc.tile_pool(name="stat", bufs=1) as stat_pool,
        ):
            acc = stat_pool.tile([P, out_cols], mybir.dt.float32)
            col_counter = [0]

            def compute(s_ap, o_ap, x_ap, fc):
                col = col_counter[0]
                col_counter[0] += 1
                eq = work_pool.tile([P, fc], mybir.dt.float32, tag="eq")
                nc.vector.tensor_tensor(
                    out=eq[:], in0=s_ap, in1=o_ap, op=mybir.AluOpType.is_equal
                )
                nc.vector.scalar_tensor_tensor(
                    out=eq[:],
                    in0=eq[:],
                    scalar=-0.5,
                    in1=x_ap,
                    op0=mybir.AluOpType.add,
                    op1=mybir.AluOpType.mult,
                    accum_out=acc[:, col : col + 1],
                )

            for t in range(T):
                base = 3 * t * P * F
                if t == 0 or t == T - 1:
                    # Tapered edge tiles: small sub-DMAs so the pipeline
                    # fills fast at the start and the last compute quantum
                    # gates on a small DMA at the end. The outermost quanta
                    # are halved again (256 KiB) to shave the very head/tail.
                    if t == 0:
                        schedule = [FC // 2, FC // 2, FC, FC, FC]
                    else:
                        schedule = [FC, FC, FC, FC // 2, FC // 2]
                    off = 0
                    for fc in schedule:
                        lo = base + off
                        off += P * fc
                        s_t = edge_pool.tile([P, FC], mybir.dt.int32, tag="se")
                        o_t = edge_pool.tile([P, FC], mybir.dt.int32, tag="oe")
                        x_t = edge_pool.tile([P, FC], mybir.dt.float32, tag="xe")
                        nc.sync.dma_start(out=s_t[:, :fc], in_=view(lo, fc))
                        nc.sync.dma_start(
                            out=o_t[:, :fc], in_=view(lo + P * F, fc)
                        )
                        nc.sync.dma_start(
                            out=x_t[:, :fc].bitcast(mybir.dt.int32),
                            in_=view(lo + 2 * P * F, fc),
                        )
                        compute(s_t[:, :fc], o_t[:, :fc], x_t[:, :fc], fc)
                else:
                    s_t = io_pool.tile([P, F], mybir.dt.int32, tag="s")
                    o_t = io_pool.tile([P, F], mybir.dt.int32, tag="o")
                    x_t = io_pool.tile([P, F], mybir.dt.float32, tag="x")
                    nc.sync.dma_start(out=s_t[:], in_=view(base, F))
                    nc.sync.dma_start(out=o_t[:], in_=view(base + P * F, F))
                    nc.sync.dma_start(
                        out=x_t[:].bitcast(mybir.dt.int32),
                        in_=view(base + 2 * P * F, F),
                    )
                    for j in range(F // FC_MID):
                        sl = slice(j * FC_MID, (j + 1) * FC_MID)
                        compute(s_t[:, sl], o_t[:, sl], x_t[:, sl], FC_MID)

            nc.sync.dma_start(out=out[:], in_=acc[:])

    nc.compile()
    _cache["nc"] = nc
    return nc


def _shard_interleaved(s, other_s, x, c):
    """Per-core buffer: for each DMA tile t, consecutive blocks
    [s_t | o_t | x_t] (each P*F int32 words; x bit-punned)."""
    sl = slice(c * PER, (c + 1) * PER)
    return np.ascontiguousarray(
        np.stack(
            [
                s[sl].reshape(T, P * F),
                other_s[sl].reshape(T, P * F),
                x[sl].view(np.int32).reshape(T, P * F),
            ],
            axis=1,
        ).reshape(3 * PER)
    )


def run(s, other_s, x, **spmd_kwargs):
    """Run on HW; returns (full_output, BassKernelResults)."""
    s = np.ascontiguousarray(np.asarray(s, dtype=np.int32).reshape(N))
    other_s = np.ascontiguousarray(np.asarray(other_s, dtype=np.int32).reshape(N))
    x = np.ascontiguousarray(np.asarray(x, dtype=np.float32).reshape(N))

    nc = _build()
    in_maps = [
        {"sox": _shard_interleaved(s, other_s, x, c)} for c in range(NCORES)
    ]
    res = run_bass_kernel_spmd(nc, in_maps, core_ids=list(range(NCORES)), **spmd_kwargs)

    total = 0.0
    for r in res.results:
        total += float(np.sum(r["out"].astype(np.float64)))
    full = np.array(2.0 * total / N, dtype=np.float32)
    return full, res


def kernel(s, other_s, x):
    out, _ = run(s, other_s, x)
    return out
